# revision 15
# baseline (speedup 1.0000x reference)
"""Trainium2 Bass kernel for nn_Decoder (latent-grid decoder MLP).

Contract: kernel(**inputs) takes the FULL unsharded inputs (as produced by
setup_inputs()) and returns the FULL [65536, 4] float32 output. Internally the
65536 points are sharded across 8 NeuronCores (pure data parallel); the small
weights are replicated.

Algorithm (mathematically equivalent to the reference):
  - G=2 trilinear interp of a per-sample 2x2x2 grid always lands in cell
    (0,0,0) (indices clip to [0, G-2] = [0,0]), so
    lat_i = sum_m w_m(xyz) * (lat @ A_m), A_m = convT_w[:, :, di, dj, dk].
  - The interp + Fourier features + first MLP layer fold into one matmul:
    u = [w_0*lat, ..., w_7*lat, sin(2 pi ang), cos(2 pi ang)]  (2304 dims),
    h0 = u @ M0 with M0 = [A_stack @ W0_top; W0_sin; W0_cos] (host-folded).
  - LayerNorm mean-subtraction folds into the weights (column centering);
    ln gamma folds in too. The per-sample rstd is deferred via LN's positive
    scale invariance: activations stay unnormalized, and gi2 (squared inverse
    scale) follows gi2' = ssq_w/512 + eps*gi2, applied once at the end.
    Requires all biases and ln_b == 0 (true for this model; a numpy fallback
    covers the general case).
  - ssq_w and the eps*gi2 term are accumulated by the TensorEngine itself
    (weighted-ones stationary operands producing a broadcast [128, S] PSUM
    tile), so no partition reductions or per-sample row ops are needed.
Activations live in [feature, sample] layout; matmuls run as fp32r (full PE
rate at N=512).
"""

import os
import numpy as np

N_CORES = 8
N_TOTAL = 65536
S_CORE = N_TOTAL // N_CORES          # 8192 samples per core
BLK = 512                            # samples per block
N_BLOCKS = S_CORE // BLK             # 16
EPS = 1e-5
N_LAYERS = 8                         # LN+relu layers (layer0 + 7 hidden)


def _precompute(inputs):
    """Host-side weight folding. Returns dict of constant arrays (fp32)."""
    convT_w = np.asarray(inputs["convT_w"], np.float32)
    W0 = np.asarray(inputs["W0"], np.float32)
    Wh = np.asarray(inputs["Wh"], np.float32)
    ln_g = np.asarray(inputs["ln_g"], np.float32)
    gauss = np.asarray(inputs["gauss"], np.float32)
    W_out = np.asarray(inputs["W_out"], np.float32)

    # A_stack[m*256+i, c] = convT_w[i, c, di, dj, dk], m = 4*di + 2*dj + dk
    A_stack = convT_w.transpose(2, 3, 4, 0, 1).reshape(8 * 256, 512)
    M0 = np.concatenate([A_stack @ W0[:512], W0[512:640], W0[640:768]], axis=0)

    def center_scale(W, g):
        Wc = W - W.mean(axis=1, keepdims=True)
        return np.ascontiguousarray(Wc * g[None, :], np.float32)

    W_eff = [center_scale(M0, ln_g[0])] + [
        center_scale(Wh[l], ln_g[l + 1]) for l in range(7)
    ]
    # pack each layer's weights as [128, n_kchunks, 512]
    def pack(W):
        K = W.shape[0]
        kc = K // 128
        return W.reshape(kc, 128, 512).transpose(1, 0, 2).reshape(128, kc * 512)

    w0p = np.ascontiguousarray(pack(W_eff[0]))                       # [128, 18*512]
    whp = np.ascontiguousarray(
        np.concatenate([pack(W) for W in W_eff[1:]], axis=1))        # [128, 28*512]
    # stats lhsT, per layer j and feature chunk mc:
    # tile[k, mc*128 + m] = 1/(512 * g_j[mc*128+k]^2)  (replicated along m)
    sw_cols = []
    for j in range(8):
        swv = (1.0 / (512.0 * ln_g[j] ** 2)).astype(np.float32)
        t = np.empty((128, 512), np.float32)
        for mc in range(4):
            t[:, mc * 128:(mc + 1) * 128] = swv[mc * 128:(mc + 1) * 128, None]
        sw_cols.append(t)
    swp = np.ascontiguousarray(np.concatenate(sw_cols, axis=1))      # [128, 8*512]

    return {
        "w0p": w0p,
        "whp": whp,
        "swp": swp,
        "epst": np.full((128, 128), EPS / 128.0, np.float32),
        "ones_row": np.ones((1, 128), np.float32),
        "ident": np.eye(128, dtype=np.float32),
        "gaussT": np.ascontiguousarray(gauss.T.astype(np.float32)),  # [3, 128]
        "sel8": np.ascontiguousarray(
            np.kron(np.eye(8, dtype=np.float32), np.ones((1, 128), np.float32))),
        "onesgi": np.ones((128, 512), np.float32),
        "woutp": np.ascontiguousarray(
            W_out.reshape(4, 128, 4).transpose(1, 0, 2).reshape(128, 16)),
    }


def _general_case_needed(inputs):
    z = lambda a: bool(np.all(np.asarray(a) == 0))
    return not (
        z(inputs["convT_b"]) and z(inputs["b0"]) and z(inputs["bh"])
        and z(inputs["ln_b"]) and z(inputs["b_out"])
        and bool(np.all(np.abs(np.asarray(inputs["ln_g"])) > 1e-3))
    )


def _numpy_fallback(inputs):
    """Reference in numpy (slow; only for inputs outside the fast path)."""
    inp = np.asarray(inputs["input"], np.float32)
    convT_w = np.asarray(inputs["convT_w"], np.float32)
    convT_b = np.asarray(inputs["convT_b"], np.float32)
    gauss = np.asarray(inputs["gauss"], np.float32)
    W0 = np.asarray(inputs["W0"], np.float32)
    b0 = np.asarray(inputs["b0"], np.float32)
    Wh = np.asarray(inputs["Wh"], np.float32)
    bh = np.asarray(inputs["bh"], np.float32)
    ln_g = np.asarray(inputs["ln_g"], np.float32)
    ln_b = np.asarray(inputs["ln_b"], np.float32)
    W_out = np.asarray(inputs["W_out"], np.float32)
    b_out = np.asarray(inputs["b_out"], np.float32)
    xyz = inp[:, -3:]
    lat = inp[:, :-3]
    f = (xyz + 1.0) * 0.5
    frac = f - np.clip(f.astype(np.int32), 0, 0)
    A = convT_w.transpose(2, 3, 4, 0, 1)
    lat_i = np.zeros((inp.shape[0], 512), np.float32)
    wx = [1 - frac[:, 0], frac[:, 0]]
    wy = [1 - frac[:, 1], frac[:, 1]]
    wz = [1 - frac[:, 2], frac[:, 2]]
    for di in (0, 1):
        for dj in (0, 1):
            for dk in (0, 1):
                w = (wx[di] * wy[dj] * wz[dk]).astype(np.float32)
                lat_i += (lat @ A[di, dj, dk]) * w[:, None]
    lat_i += convT_b[None, :]
    ang = 2 * np.pi * (xyz @ gauss.T)
    x = np.concatenate([lat_i, np.sin(ang), np.cos(ang)], axis=1)

    def ln(t, g, b):
        mu = t.mean(-1, keepdims=True)
        var = ((t - mu) ** 2).mean(-1, keepdims=True)
        return (t - mu) / np.sqrt(var + EPS) * g + b

    x = np.maximum(ln(x @ W0 + b0, ln_g[0], ln_b[0]), 0)
    for l in range(7):
        x = np.maximum(ln(x @ Wh[l] + bh[l], ln_g[l + 1], ln_b[l + 1]), 0)
    y = x @ W_out + b_out
    return np.concatenate([np.tanh(y[:, :1]), y[:, 1:] * 255.0], axis=1).astype(np.float32)


_NC_CACHE = {}


def _build_bass(s_core=S_CORE):
    """Build the per-core Bass module (SPMD; same program on all 8 cores)."""
    import concourse.bass as bass
    import concourse.bacc as bacc
    import concourse.tile as tile
    from concourse import mybir

    FP32 = mybir.dt.float32
    FP32R = mybir.dt.float32r
    AF = mybir.ActivationFunctionType
    ALU = mybir.AluOpType
    TWO_PI = float(2.0 * np.pi)
    MAGIC = 12582912.0            # 1.5 * 2^23: fp32 add/sub rounds to integer
    n_blocks = s_core // BLK

    nc = bacc.Bacc("TRN2", target_bir_lowering=False, debug=False)

    inp_d = nc.dram_tensor("inp", [s_core, 259], FP32, kind="ExternalInput").ap()
    w0p_d = nc.dram_tensor("w0p", [128, 18 * 512], FP32R, kind="ExternalInput").ap()
    whp_d = nc.dram_tensor("whp", [128, 28 * 512], FP32R, kind="ExternalInput").ap()
    swp_d = nc.dram_tensor("swp", [128, 8 * 512], FP32R, kind="ExternalInput").ap()
    epst_d = nc.dram_tensor("epst", [128, 128], FP32R, kind="ExternalInput").ap()
    ones_d = nc.dram_tensor("ones_row", [1, 128], FP32, kind="ExternalInput").ap()
    ident_d = nc.dram_tensor("ident", [128, 128], FP32, kind="ExternalInput").ap()
    gaussT_d = nc.dram_tensor("gaussT", [3, 128], FP32R, kind="ExternalInput").ap()
    sel8_d = nc.dram_tensor("sel8", [8, 8 * 128], FP32R, kind="ExternalInput").ap()
    onesgi_d = nc.dram_tensor("onesgi", [128, 512], FP32R, kind="ExternalInput").ap()
    woutp_d = nc.dram_tensor("woutp", [128, 16], FP32R, kind="ExternalInput").ap()
    outT_d = nc.dram_tensor("outT", [4, s_core], FP32, kind="ExternalOutput").ap()

    def r(ap):
        return ap

    with tile.TileContext(nc) as tc:
        with (
            tc.tile_pool(name="const", bufs=1) as constp,
            tc.tile_pool(name="weights", bufs=1) as weightp,
            tc.tile_pool(name="inblk", bufs=2) as inp_pool,
            tc.tile_pool(name="acts", bufs=2) as actp,
            tc.tile_pool(name="scratch", bufs=2) as scr,
            tc.tile_pool(name="fin", bufs=2) as finp,
            tc.tile_pool(name="dram", bufs=1, space="DRAM") as dramp,
            tc.tile_pool(name="ps_t", bufs=1, space="PSUM") as ps_t,
            tc.tile_pool(name="ps_misc", bufs=2, space="PSUM") as ps_misc,
            tc.tile_pool(name="ps_gi", bufs=2, space="PSUM") as ps_gi,
        ):
            # ---- constants / weights (loaded once, resident) ----
            w0_sb = weightp.tile([128, 18, 512], FP32R)
            nc.sync.dma_start(out=w0_sb, in_=w0p_d.rearrange("p (c f) -> p c f", c=18))
            wh_sb = weightp.tile([128, 28, 512], FP32R)
            nc.sync.dma_start(out=wh_sb, in_=whp_d.rearrange("p (c f) -> p c f", c=28))
            sw_sb = weightp.tile([128, 8, 512], FP32R)
            nc.sync.dma_start(out=sw_sb, in_=swp_d.rearrange("p (c f) -> p c f", c=8))
            eps_sb = constp.tile([128, 128], FP32R)
            nc.sync.dma_start(out=eps_sb, in_=epst_d)
            ones_sb = constp.tile([1, 128], FP32)
            nc.sync.dma_start(out=ones_sb, in_=ones_d)
            ident_dma = constp.tile([128, 128], FP32, name="ident_dma")
            nc.sync.dma_start(out=ident_dma, in_=ident_d)
            # DVE-gate the identity so PE transposes only ever wait on DVE
            ident_sb = constp.tile([128, 128], FP32, name="ident_sb")
            nc.vector.tensor_copy(ident_sb, ident_dma)
            gaussT_sb = constp.tile([3, 128], FP32R)
            nc.sync.dma_start(out=gaussT_sb, in_=gaussT_d)
            sel8_sb = constp.tile([8, 8, 128], FP32R)
            nc.sync.dma_start(out=sel8_sb, in_=sel8_d.rearrange("p (m f) -> p m f", m=8))
            wout_sb = weightp.tile([128, 4, 4], FP32R)
            nc.sync.dma_start(out=wout_sb, in_=woutp_d.rearrange("p (c f) -> p c f", c=4))
            ones_gi_sb = constp.tile([128, BLK], FP32R)
            nc.sync.dma_start(out=ones_gi_sb, in_=onesgi_d)

            # DRAM bounce for the final (whole-core) normalization stage
            yhat_dr = dramp.tile([4, s_core], FP32)
            gi2_dr = dramp.tile([4, s_core], FP32)

            inp_r = inp_d.rearrange("(b sc p) f -> b p sc f", sc=4, p=128)

            for b in range(n_blocks):
                # ---- load input block; [128p, 4sc, 259f] (sample-major) ----
                inb0 = inp_pool.tile([128, 4, 259], FP32, tag="inb0")
                nc.sync.dma_start(out=inb0, in_=inp_r[b])
                # DVE-gate the input block: all downstream consumers then
                # depend on the DVE semaphore only (PE LDW takes 1 wait max)
                inb = inp_pool.tile([128, 4, 259], FP32, tag="inb")
                nc.vector.tensor_copy(inb, inb0)

                # ---- transpose lat and xyz to [feature, sample] ----
                latT = scr.tile([128, 2, BLK], FP32, tag="latT", bufs=1)
                xyzT = scr.tile([3, BLK], FP32R, tag="xyzT")
                for sc in range(4):
                    for fc in range(2):
                        tp = ps_misc.tile([128, 128], FP32, tag="mt")
                        nc.tensor.transpose(
                            tp, inb[:, sc, fc * 128:(fc + 1) * 128], ident_sb
                        )
                        nc.vector.tensor_copy(latT[:, fc, sc * 128:(sc + 1) * 128], tp)
                    tp3 = ps_misc.tile([3, 128], FP32, tag="mt")
                    nc.tensor.transpose(tp3, inb[:, sc, 256:259], ident_sb)
                    nc.vector.tensor_copy(xyzT[:, sc * 128:(sc + 1) * 128], tp3)

                # ---- corner weights in sample layout, then transpose ----
                w8T = scr.tile([8, BLK], FP32R, tag="w8T", bufs=1)
                for sc in range(4):
                    f3 = scr.tile([128, 3], FP32, tag="f3")
                    nc.vector.tensor_scalar(
                        out=f3, in0=inb[:, sc, 256:259],
                        scalar1=0.5, scalar2=0.5, op0=ALU.mult, op1=ALU.add,
                    )
                    om3 = scr.tile([128, 3], FP32, tag="om3")
                    nc.vector.tensor_scalar(
                        out=om3, in0=f3, scalar1=1.0, scalar2=-1.0,
                        op0=ALU.subtract, op1=ALU.mult,
                    )
                    wxy = scr.tile([128, 4], FP32, tag="wxy")
                    w8s = scr.tile([128, 8], FP32, tag="w8s")
                    for di in (0, 1):
                        xs = (f3 if di else om3)[:, 0:1]
                        for dj in (0, 1):
                            ys = (f3 if dj else om3)[:, 1:2]
                            nc.vector.tensor_tensor(
                                out=wxy[:, di * 2 + dj:di * 2 + dj + 1],
                                in0=xs, in1=ys, op=ALU.mult,
                            )
                    for m in range(8):
                        di, dj, dk = (m >> 2) & 1, (m >> 1) & 1, m & 1
                        zsl = (f3 if dk else om3)[:, 2:3]
                        nc.vector.tensor_tensor(
                            out=w8s[:, m:m + 1],
                            in0=wxy[:, di * 2 + dj:di * 2 + dj + 1],
                            in1=zsl, op=ALU.mult,
                        )
                    tp8 = ps_misc.tile([8, 128], FP32, tag="mt")
                    nc.tensor.transpose(tp8, w8s, ident_sb)
                    nc.vector.tensor_copy(w8T[:, sc * 128:(sc + 1) * 128], tp8)

                # ---- fourier angle, range-reduced to [-0.5, 0.5] ----
                angp = ps_misc.tile([128, BLK], FP32, tag="mt")
                nc.tensor.matmul(angp, r(gaussT_sb), r(xyzT), start=True, stop=True)
                ang_sb = scr.tile([128, BLK], FP32, tag="rr", bufs=3, name="ang_sb")
                nc.vector.tensor_copy(ang_sb, angp)
                # zs = ang - round(ang); zc = a25 - round(a25), a25 = ang + 0.25
                zs_r = scr.tile([128, BLK], FP32, tag="rr", bufs=3, name="zs_r")
                nc.vector.tensor_scalar(
                    out=zs_r, in0=ang_sb, scalar1=MAGIC, scalar2=MAGIC,
                    op0=ALU.add, op1=ALU.subtract,
                )
                zs = scr.tile([128, BLK], FP32, tag="zs", bufs=1)
                nc.vector.tensor_sub(zs, ang_sb, zs_r)
                a25 = scr.tile([128, BLK], FP32, tag="a25", bufs=1)
                nc.vector.tensor_scalar_add(out=a25, in0=ang_sb, scalar1=0.25)
                zc_r = scr.tile([128, BLK], FP32, tag="rr", bufs=3, name="zc_r")
                nc.vector.tensor_scalar(
                    out=zc_r, in0=a25, scalar1=MAGIC, scalar2=MAGIC,
                    op0=ALU.add, op1=ALU.subtract,
                )
                zc = scr.tile([128, BLK], FP32, tag="zc", bufs=1)
                nc.vector.tensor_sub(zc, a25, zc_r)

                # ---- layer 0: build u chunks incrementally + matmul ----
                psums = [ps_t.tile([128, BLK], FP32, tag=f"pt{mc}", name=f"pt{mc}") for mc in range(4)]
                uch_i = 0

                def l0_accum(u_ap, last=False):
                    nonlocal uch_i
                    for mc in range(4):
                        nc.tensor.matmul(
                            psums[mc],
                            r(w0_sb[:, uch_i, mc * 128:(mc + 1) * 128]),
                            r(u_ap),
                            start=(uch_i == 0), stop=last,
                        )
                    uch_i += 1

                for m in range(8):
                    bc = ps_misc.tile([128, BLK], FP32, tag="mt")
                    nc.tensor.matmul(
                        bc, r(sel8_sb[:, m, :]), r(w8T), start=True, stop=True
                    )
                    wbm = scr.tile([128, BLK], FP32, tag="wbm")
                    nc.vector.tensor_copy(wbm, bc)
                    for kc in range(2):
                        uch = scr.tile([128, BLK], FP32R, tag="uch")
                        nc.vector.tensor_tensor(
                            out=uch, in0=latT[:, kc, :], in1=wbm, op=ALU.mult
                        )
                        l0_accum(uch)
                ffs = scr.tile([128, BLK], FP32R, tag="uch")
                nc.scalar.activation(out=ffs, in_=zs, func=AF.Sin, scale=TWO_PI)
                l0_accum(ffs)
                ffc = scr.tile([128, BLK], FP32R, tag="uch")
                nc.scalar.activation(out=ffc, in_=zc, func=AF.Sin, scale=TWO_PI)
                l0_accum(ffc, last=True)

                # ---- LN layers: stats + relu; then hidden matmuls ----
                gi2_cur = ones_gi_sb
                x_cur = None
                for j in range(N_LAYERS):
                    if j > 0:
                        psums = [
                            ps_t.tile([128, BLK], FP32, tag=f"pt{mc}", name=f"pt{mc}")
                            for mc in range(4)
                        ]
                        for mc in range(4):
                            for kc in range(4):
                                nc.tensor.matmul(
                                    psums[mc],
                                    r(wh_sb[:, (j - 1) * 4 + kc,
                                            mc * 128:(mc + 1) * 128]),
                                    r(x_cur[:, kc, :]),
                                    start=(kc == 0), stop=(kc == 3),
                                )
                    sq = scr.tile([128, 4, BLK], FP32R, tag="sq", bufs=1)
                    x_next = actp.tile([128, 4, BLK], FP32R, tag="xn")
                    gp = ps_gi.tile([128, BLK], FP32, tag="gp")
                    for mc in range(4):
                        nc.scalar.activation(
                            out=sq[:, mc, :], in_=psums[mc], func=AF.Square
                        )
                    for mc in range(4):
                        nc.tensor.matmul(
                            gp, r(sw_sb[:, j, mc * 128:(mc + 1) * 128]),
                            r(sq[:, mc, :]), start=(mc == 0), stop=False,
                        )
                    nc.tensor.matmul(gp, r(eps_sb), r(gi2_cur), start=False, stop=True)
                    gi2_new = scr.tile([128, BLK], FP32R, tag="gi2", bufs=2)
                    nc.vector.tensor_copy(gi2_new, gp)
                    for mc in range(4):
                        if mc % 2 == 0:
                            nc.scalar.activation(
                                out=x_next[:, mc, :], in_=psums[mc], func=AF.Relu
                            )
                        else:
                            nc.vector.tensor_scalar_max(
                                out=x_next[:, mc, :], in0=psums[mc], scalar1=0.0
                            )
                    x_cur = x_next
                    gi2_cur = gi2_new

                # ---- output layer ----
                yp = ps_misc.tile([4, BLK], FP32, tag="mt")
                for kc in range(4):
                    nc.tensor.matmul(
                        yp, r(wout_sb[:, kc, :]), r(x_cur[:, kc, :]),
                        start=(kc == 0), stop=(kc == 3),
                    )
                ysb = scr.tile([4, BLK], FP32, tag="ysb")
                nc.vector.tensor_copy(ysb, yp)
                nc.sync.dma_start(out=yhat_dr[:, b * BLK:(b + 1) * BLK], in_=ysb)
                g4 = scr.tile([4, BLK], FP32, tag="g4")
                nc.vector.tensor_copy(g4, gi2_cur[0:4, :])
                nc.sync.dma_start(out=gi2_dr[:, b * BLK:(b + 1) * BLK], in_=g4)

            # ---- final: y = yhat / sqrt(gi2); out = [tanh(y0), 255*y1..3] ----
            # Two passes bounced through DRAM so the ACT table set only
            # switches twice (Sqrt set in pass A, back to the sin/tanh set).
            FB = min(512, s_core)
            for p in range(s_core // FB):
                sl = slice(p * FB, (p + 1) * FB)
                yh = finp.tile([4, FB], FP32, tag="yh", name="yh")
                gi = finp.tile([4, FB], FP32, tag="gi", name="gi")
                nc.sync.dma_start(out=yh, in_=yhat_dr[:, sl])
                nc.sync.dma_start(out=gi, in_=gi2_dr[:, sl])
                nc.scalar.activation(out=gi, in_=gi, func=AF.Sqrt)
                nc.vector.reciprocal(out=gi, in_=gi)
                nc.vector.tensor_mul(yh, yh, gi)
                nc.sync.dma_start(out=yhat_dr[:, sl], in_=yh)
            for p in range(s_core // FB):
                sl = slice(p * FB, (p + 1) * FB)
                yh = finp.tile([4, FB], FP32, tag="yh", name="yh")
                nc.sync.dma_start(out=yh, in_=yhat_dr[:, sl])
                nc.scalar.mul(out=yh, in_=yh, mul=255.0)
                nc.scalar.activation(
                    out=yh[0:1, :], in_=yh[0:1, :], func=AF.Tanh, scale=1.0 / 255.0
                )
                nc.sync.dma_start(out=outT_d[:, sl], in_=yh)

    nc.compile()
    return nc


def kernel(**inputs):
    if _general_case_needed(inputs):
        return _numpy_fallback(inputs)

    from concourse.bass_utils import run_bass_kernel_spmd

    pre = _precompute(inputs)
    inp = np.ascontiguousarray(np.asarray(inputs["input"], np.float32))

    if "nc" not in _NC_CACHE:
        _NC_CACHE["nc"] = _build_bass()
    nc = _NC_CACHE["nc"]

    in_maps = [
        {
            "inp": np.ascontiguousarray(inp[c * S_CORE:(c + 1) * S_CORE]),
            "w0p": pre["w0p"], "whp": pre["whp"], "swp": pre["swp"],
            "epst": pre["epst"], "ones_row": pre["ones_row"],
            "ident": pre["ident"], "gaussT": pre["gaussT"],
            "sel8": pre["sel8"], "woutp": pre["woutp"],
            "onesgi": pre["onesgi"],
        }
        for c in range(N_CORES)
    ]

    res = run_bass_kernel_spmd(
        nc, in_maps, core_ids=list(range(N_CORES)),
        trace=bool(int(os.environ.get("KERNEL_TRACE", "0"))),
    )
    kernel.last_results = res
    outs = [res.results[c]["outT"] for c in range(N_CORES)]
    return np.ascontiguousarray(
        np.concatenate([o.T for o in outs], axis=0).astype(np.float32)
    )


# revision 16
# speedup vs baseline: 68.5909x; 68.5909x over previous
"""Trainium2 Bass kernel for nn_Decoder (latent-grid decoder MLP).

Contract: kernel(**inputs) takes the FULL unsharded inputs (as produced by
setup_inputs()) and returns the FULL [65536, 4] float32 output. Internally the
65536 points are sharded across 8 NeuronCores (pure data parallel); the small
weights are replicated.

Algorithm (mathematically equivalent to the reference):
  - G=2 trilinear interp of a per-sample 2x2x2 grid always lands in cell
    (0,0,0) (indices clip to [0, G-2] = [0,0]), so
    lat_i = sum_m w_m(xyz) * (lat @ A_m), A_m = convT_w[:, :, di, dj, dk].
  - The interp + Fourier features + first MLP layer fold into one matmul:
    u = [w_0*lat, ..., w_7*lat, sin(2 pi ang), cos(2 pi ang)]  (2304 dims),
    h0 = u @ M0 with M0 = [A_stack @ W0_top; W0_sin; W0_cos] (host-folded).
  - LayerNorm mean-subtraction folds into the weights (column centering);
    ln gamma folds in too. The per-sample rstd is deferred via LN's positive
    scale invariance: activations stay unnormalized, and gi2 (squared inverse
    scale) follows gi2' = ssq_w/512 + eps*gi2, applied once at the end.
    Requires all biases and ln_b == 0 (true for this model; a numpy fallback
    covers the general case).
  - ssq_w and the eps*gi2 term are accumulated by the TensorEngine itself
    (weighted-ones stationary operands producing a broadcast [128, S] PSUM
    tile), so no partition reductions or per-sample row ops are needed.
Activations live in [feature, sample] layout; matmuls run as fp32r (full PE
rate at N=512).
"""

import os
import numpy as np

N_CORES = 8
N_TOTAL = 65536
S_CORE = N_TOTAL // N_CORES          # 8192 samples per core
BLK = 512                            # samples per block
N_BLOCKS = S_CORE // BLK             # 16
EPS = 1e-5
N_LAYERS = 8                         # LN+relu layers (layer0 + 7 hidden)


def _precompute(inputs):
    """Host-side weight folding. Returns dict of constant arrays (fp32)."""
    convT_w = np.asarray(inputs["convT_w"], np.float32)
    W0 = np.asarray(inputs["W0"], np.float32)
    Wh = np.asarray(inputs["Wh"], np.float32)
    ln_g = np.asarray(inputs["ln_g"], np.float32)
    gauss = np.asarray(inputs["gauss"], np.float32)
    W_out = np.asarray(inputs["W_out"], np.float32)

    # A_stack[m*256+i, c] = convT_w[i, c, di, dj, dk], m = 4*di + 2*dj + dk
    A_stack = convT_w.transpose(2, 3, 4, 0, 1).reshape(8 * 256, 512)
    M0 = np.concatenate([A_stack @ W0[:512], W0[512:640], W0[640:768]], axis=0)

    def center_scale(W, g):
        Wc = W - W.mean(axis=1, keepdims=True)
        return np.ascontiguousarray(Wc * g[None, :], np.float32)

    W_eff = [center_scale(M0, ln_g[0])] + [
        center_scale(Wh[l], ln_g[l + 1]) for l in range(7)
    ]
    # pack each layer's weights as [128, n_kchunks, 512]
    def pack(W):
        K = W.shape[0]
        kc = K // 128
        return W.reshape(kc, 128, 512).transpose(1, 0, 2).reshape(128, kc * 512)

    w0p = np.ascontiguousarray(pack(W_eff[0]))                       # [128, 18*512]
    whp = np.ascontiguousarray(
        np.concatenate([pack(W) for W in W_eff[1:]], axis=1))        # [128, 28*512]
    # stats lhsT, per layer j and feature chunk mc:
    # tile[k, mc*128 + m] = 1/(512 * g_j[mc*128+k]^2)  (replicated along m)
    sw_cols = []
    for j in range(8):
        swv = (1.0 / (512.0 * ln_g[j] ** 2)).astype(np.float32)
        t = np.empty((128, 512), np.float32)
        for mc in range(4):
            t[:, mc * 128:(mc + 1) * 128] = swv[mc * 128:(mc + 1) * 128, None]
        sw_cols.append(t)
    swp = np.ascontiguousarray(np.concatenate(sw_cols, axis=1))      # [128, 8*512]

    return {
        "w0p": w0p,
        "whp": whp,
        "swp": swp,
        "epst": np.full((128, 128), EPS / 128.0, np.float32),
        "ones_row": np.ones((1, 128), np.float32),
        "ident": np.eye(128, dtype=np.float32),
        "gaussT": np.ascontiguousarray(gauss.T.astype(np.float32)),  # [3, 128]
        "sel8": np.ascontiguousarray(
            np.kron(np.eye(8, dtype=np.float32), np.ones((1, 128), np.float32))),
        "onesgi": np.ones((128, 512), np.float32),
        "woutp": np.ascontiguousarray(
            W_out.reshape(4, 128, 4).transpose(1, 0, 2).reshape(128, 16)),
    }


def _general_case_needed(inputs):
    z = lambda a: bool(np.all(np.asarray(a) == 0))
    return not (
        z(inputs["convT_b"]) and z(inputs["b0"]) and z(inputs["bh"])
        and z(inputs["ln_b"]) and z(inputs["b_out"])
        and bool(np.all(np.abs(np.asarray(inputs["ln_g"])) > 1e-3))
    )


def _numpy_fallback(inputs):
    """Reference in numpy (slow; only for inputs outside the fast path)."""
    inp = np.asarray(inputs["input"], np.float32)
    convT_w = np.asarray(inputs["convT_w"], np.float32)
    convT_b = np.asarray(inputs["convT_b"], np.float32)
    gauss = np.asarray(inputs["gauss"], np.float32)
    W0 = np.asarray(inputs["W0"], np.float32)
    b0 = np.asarray(inputs["b0"], np.float32)
    Wh = np.asarray(inputs["Wh"], np.float32)
    bh = np.asarray(inputs["bh"], np.float32)
    ln_g = np.asarray(inputs["ln_g"], np.float32)
    ln_b = np.asarray(inputs["ln_b"], np.float32)
    W_out = np.asarray(inputs["W_out"], np.float32)
    b_out = np.asarray(inputs["b_out"], np.float32)
    xyz = inp[:, -3:]
    lat = inp[:, :-3]
    f = (xyz + 1.0) * 0.5
    frac = f - np.clip(f.astype(np.int32), 0, 0)
    A = convT_w.transpose(2, 3, 4, 0, 1)
    lat_i = np.zeros((inp.shape[0], 512), np.float32)
    wx = [1 - frac[:, 0], frac[:, 0]]
    wy = [1 - frac[:, 1], frac[:, 1]]
    wz = [1 - frac[:, 2], frac[:, 2]]
    for di in (0, 1):
        for dj in (0, 1):
            for dk in (0, 1):
                w = (wx[di] * wy[dj] * wz[dk]).astype(np.float32)
                lat_i += (lat @ A[di, dj, dk]) * w[:, None]
    lat_i += convT_b[None, :]
    ang = 2 * np.pi * (xyz @ gauss.T)
    x = np.concatenate([lat_i, np.sin(ang), np.cos(ang)], axis=1)

    def ln(t, g, b):
        mu = t.mean(-1, keepdims=True)
        var = ((t - mu) ** 2).mean(-1, keepdims=True)
        return (t - mu) / np.sqrt(var + EPS) * g + b

    x = np.maximum(ln(x @ W0 + b0, ln_g[0], ln_b[0]), 0)
    for l in range(7):
        x = np.maximum(ln(x @ Wh[l] + bh[l], ln_g[l + 1], ln_b[l + 1]), 0)
    y = x @ W_out + b_out
    return np.concatenate([np.tanh(y[:, :1]), y[:, 1:] * 255.0], axis=1).astype(np.float32)


_NC_CACHE = {}


def _build_bass(s_core=S_CORE, repeat=1):
    """Build the per-core Bass module (SPMD; same program on all 8 cores)."""
    import concourse.bass as bass
    import concourse.bacc as bacc
    import concourse.tile as tile
    from concourse import mybir

    FP32 = mybir.dt.float32
    FP32R = mybir.dt.float32r
    AF = mybir.ActivationFunctionType
    ALU = mybir.AluOpType
    TWO_PI = float(2.0 * np.pi)
    MAGIC = 12582912.0            # 1.5 * 2^23: fp32 add/sub rounds to integer
    n_blocks = s_core // BLK

    nc = bacc.Bacc("TRN2", target_bir_lowering=False, debug=False)

    inp_d = nc.dram_tensor("inp", [s_core, 259], FP32, kind="ExternalInput").ap()
    w0p_d = nc.dram_tensor("w0p", [128, 18 * 512], FP32R, kind="ExternalInput").ap()
    whp_d = nc.dram_tensor("whp", [128, 28 * 512], FP32R, kind="ExternalInput").ap()
    swp_d = nc.dram_tensor("swp", [128, 8 * 512], FP32R, kind="ExternalInput").ap()
    epst_d = nc.dram_tensor("epst", [128, 128], FP32R, kind="ExternalInput").ap()
    ones_d = nc.dram_tensor("ones_row", [1, 128], FP32, kind="ExternalInput").ap()
    ident_d = nc.dram_tensor("ident", [128, 128], FP32, kind="ExternalInput").ap()
    gaussT_d = nc.dram_tensor("gaussT", [3, 128], FP32R, kind="ExternalInput").ap()
    sel8_d = nc.dram_tensor("sel8", [8, 8 * 128], FP32R, kind="ExternalInput").ap()
    onesgi_d = nc.dram_tensor("onesgi", [128, 512], FP32R, kind="ExternalInput").ap()
    woutp_d = nc.dram_tensor("woutp", [128, 16], FP32R, kind="ExternalInput").ap()
    outT_d = nc.dram_tensor("outT", [4, s_core], FP32, kind="ExternalOutput").ap()

    def r(ap):
        return ap

    with tile.TileContext(nc) as tc:
        with (
            tc.tile_pool(name="const", bufs=1) as constp,
            tc.tile_pool(name="weights", bufs=1) as weightp,
            tc.tile_pool(name="inblk", bufs=2) as inp_pool,
            tc.tile_pool(name="acts", bufs=2) as actp,
            tc.tile_pool(name="scratch", bufs=2) as scr,
            tc.tile_pool(name="fin", bufs=2) as finp,
            tc.tile_pool(name="dram", bufs=1, space="DRAM") as dramp,
            tc.tile_pool(name="ps_t", bufs=1, space="PSUM") as ps_t,
            tc.tile_pool(name="ps_misc", bufs=2, space="PSUM") as ps_misc,
            tc.tile_pool(name="ps_gi", bufs=2, space="PSUM") as ps_gi,
        ):
            # ---- constants / weights (loaded once, resident) ----
            w0_sb = weightp.tile([128, 18, 512], FP32R)
            nc.sync.dma_start(out=w0_sb, in_=w0p_d.rearrange("p (c f) -> p c f", c=18))
            wh_sb = weightp.tile([128, 28, 512], FP32R)
            nc.sync.dma_start(out=wh_sb, in_=whp_d.rearrange("p (c f) -> p c f", c=28))
            sw_sb = weightp.tile([128, 8, 512], FP32R)
            nc.sync.dma_start(out=sw_sb, in_=swp_d.rearrange("p (c f) -> p c f", c=8))
            eps_sb = constp.tile([128, 128], FP32R)
            nc.sync.dma_start(out=eps_sb, in_=epst_d)
            ones_sb = constp.tile([1, 128], FP32)
            nc.sync.dma_start(out=ones_sb, in_=ones_d)
            ident_dma = constp.tile([128, 128], FP32, name="ident_dma")
            nc.sync.dma_start(out=ident_dma, in_=ident_d)
            # DVE-gate the identity so PE transposes only ever wait on DVE
            ident_sb = constp.tile([128, 128], FP32, name="ident_sb")
            nc.vector.tensor_copy(ident_sb, ident_dma)
            gaussT_sb = constp.tile([3, 128], FP32R)
            nc.sync.dma_start(out=gaussT_sb, in_=gaussT_d)
            sel8_sb = constp.tile([8, 8, 128], FP32R)
            nc.sync.dma_start(out=sel8_sb, in_=sel8_d.rearrange("p (m f) -> p m f", m=8))
            wout_sb = weightp.tile([128, 4, 4], FP32R)
            nc.sync.dma_start(out=wout_sb, in_=woutp_d.rearrange("p (c f) -> p c f", c=4))
            ones_gi_sb = constp.tile([128, BLK], FP32R)
            nc.sync.dma_start(out=ones_gi_sb, in_=onesgi_d)

            # DRAM bounce for the final (whole-core) normalization stage
            yhat_dr = dramp.tile([4, s_core], FP32)
            gi2_dr = dramp.tile([4, s_core], FP32)

            inp_r = inp_d.rearrange("(b sc p) f -> b p sc f", sc=4, p=128)

            for _rep in range(repeat):
              for b in range(n_blocks):
                # ---- load input block; [128p, 4sc, 259f] (sample-major) ----
                inb0 = inp_pool.tile([128, 4, 259], FP32, tag="inb0")
                nc.sync.dma_start(out=inb0, in_=inp_r[b])
                # DVE-gate the input block: all downstream consumers then
                # depend on the DVE semaphore only (PE LDW takes 1 wait max)
                inb = inp_pool.tile([128, 4, 259], FP32, tag="inb")
                nc.vector.tensor_copy(inb, inb0)

                # ---- transpose lat and xyz to [feature, sample] ----
                latT = scr.tile([128, 2, BLK], FP32, tag="latT", bufs=1)
                xyzT = scr.tile([3, BLK], FP32R, tag="xyzT")
                for sc in range(4):
                    for fc in range(2):
                        tp = ps_misc.tile([128, 128], FP32, tag="mt")
                        nc.tensor.transpose(
                            tp, inb[:, sc, fc * 128:(fc + 1) * 128], ident_sb
                        )
                        nc.vector.tensor_copy(latT[:, fc, sc * 128:(sc + 1) * 128], tp)
                    tp3 = ps_misc.tile([3, 128], FP32, tag="mt")
                    nc.tensor.transpose(tp3, inb[:, sc, 256:259], ident_sb)
                    nc.vector.tensor_copy(xyzT[:, sc * 128:(sc + 1) * 128], tp3)

                # ---- corner weights in sample layout, then transpose ----
                w8T = scr.tile([8, BLK], FP32R, tag="w8T", bufs=1)
                for sc in range(4):
                    f3 = scr.tile([128, 3], FP32, tag="f3")
                    nc.vector.tensor_scalar(
                        out=f3, in0=inb[:, sc, 256:259],
                        scalar1=0.5, scalar2=0.5, op0=ALU.mult, op1=ALU.add,
                    )
                    om3 = scr.tile([128, 3], FP32, tag="om3")
                    nc.vector.tensor_scalar(
                        out=om3, in0=f3, scalar1=1.0, scalar2=-1.0,
                        op0=ALU.subtract, op1=ALU.mult,
                    )
                    wxy = scr.tile([128, 4], FP32, tag="wxy")
                    w8s = scr.tile([128, 8], FP32, tag="w8s")
                    for di in (0, 1):
                        xs = (f3 if di else om3)[:, 0:1]
                        for dj in (0, 1):
                            ys = (f3 if dj else om3)[:, 1:2]
                            nc.vector.tensor_tensor(
                                out=wxy[:, di * 2 + dj:di * 2 + dj + 1],
                                in0=xs, in1=ys, op=ALU.mult,
                            )
                    for m in range(8):
                        di, dj, dk = (m >> 2) & 1, (m >> 1) & 1, m & 1
                        zsl = (f3 if dk else om3)[:, 2:3]
                        nc.vector.tensor_tensor(
                            out=w8s[:, m:m + 1],
                            in0=wxy[:, di * 2 + dj:di * 2 + dj + 1],
                            in1=zsl, op=ALU.mult,
                        )
                    tp8 = ps_misc.tile([8, 128], FP32, tag="mt")
                    nc.tensor.transpose(tp8, w8s, ident_sb)
                    nc.vector.tensor_copy(w8T[:, sc * 128:(sc + 1) * 128], tp8)

                # ---- fourier angle, range-reduced to [-0.5, 0.5] ----
                angp = ps_misc.tile([128, BLK], FP32, tag="mt")
                nc.tensor.matmul(angp, r(gaussT_sb), r(xyzT), start=True, stop=True)
                ang_sb = scr.tile([128, BLK], FP32, tag="rr", bufs=3, name="ang_sb")
                nc.vector.tensor_copy(ang_sb, angp)
                # zs = ang - round(ang); zc = a25 - round(a25), a25 = ang + 0.25
                zs_r = scr.tile([128, BLK], FP32, tag="rr", bufs=3, name="zs_r")
                nc.vector.tensor_scalar(
                    out=zs_r, in0=ang_sb, scalar1=MAGIC, scalar2=MAGIC,
                    op0=ALU.add, op1=ALU.subtract,
                )
                zs = scr.tile([128, BLK], FP32, tag="zs", bufs=1)
                nc.vector.tensor_sub(zs, ang_sb, zs_r)
                a25 = scr.tile([128, BLK], FP32, tag="a25", bufs=1)
                nc.vector.tensor_scalar_add(out=a25, in0=ang_sb, scalar1=0.25)
                zc_r = scr.tile([128, BLK], FP32, tag="rr", bufs=3, name="zc_r")
                nc.vector.tensor_scalar(
                    out=zc_r, in0=a25, scalar1=MAGIC, scalar2=MAGIC,
                    op0=ALU.add, op1=ALU.subtract,
                )
                zc = scr.tile([128, BLK], FP32, tag="zc", bufs=1)
                nc.vector.tensor_sub(zc, a25, zc_r)

                # ---- layer 0: build u chunks incrementally + matmul ----
                psums = [ps_t.tile([128, BLK], FP32, tag=f"pt{mc}", name=f"pt{mc}") for mc in range(4)]
                uch_i = 0

                def l0_accum(u_ap, last=False):
                    nonlocal uch_i
                    for mc in range(4):
                        nc.tensor.matmul(
                            psums[mc],
                            r(w0_sb[:, uch_i, mc * 128:(mc + 1) * 128]),
                            r(u_ap),
                            start=(uch_i == 0), stop=last,
                        )
                    uch_i += 1

                for m in range(8):
                    bc = ps_misc.tile([128, BLK], FP32, tag="mt")
                    nc.tensor.matmul(
                        bc, r(sel8_sb[:, m, :]), r(w8T), start=True, stop=True
                    )
                    wbm = scr.tile([128, BLK], FP32, tag="wbm")
                    nc.vector.tensor_copy(wbm, bc)
                    for kc in range(2):
                        uch = scr.tile([128, BLK], FP32R, tag="uch")
                        nc.vector.tensor_tensor(
                            out=uch, in0=latT[:, kc, :], in1=wbm, op=ALU.mult
                        )
                        l0_accum(uch)
                ffs = scr.tile([128, BLK], FP32R, tag="uch")
                nc.scalar.activation(out=ffs, in_=zs, func=AF.Sin, scale=TWO_PI)
                l0_accum(ffs)
                ffc = scr.tile([128, BLK], FP32R, tag="uch")
                nc.scalar.activation(out=ffc, in_=zc, func=AF.Sin, scale=TWO_PI)
                l0_accum(ffc, last=True)

                # ---- LN layers: stats + relu; then hidden matmuls ----
                gi2_cur = ones_gi_sb
                x_cur = None
                for j in range(N_LAYERS):
                    if j > 0:
                        psums = [
                            ps_t.tile([128, BLK], FP32, tag=f"pt{mc}", name=f"pt{mc}")
                            for mc in range(4)
                        ]
                        for mc in range(4):
                            for kc in range(4):
                                nc.tensor.matmul(
                                    psums[mc],
                                    r(wh_sb[:, (j - 1) * 4 + kc,
                                            mc * 128:(mc + 1) * 128]),
                                    r(x_cur[:, kc, :]),
                                    start=(kc == 0), stop=(kc == 3),
                                )
                    sq = scr.tile([128, 4, BLK], FP32R, tag="sq", bufs=1)
                    x_next = actp.tile([128, 4, BLK], FP32R, tag="xn")
                    gp = ps_gi.tile([128, BLK], FP32, tag="gp")
                    for mc in range(4):
                        nc.scalar.activation(
                            out=sq[:, mc, :], in_=psums[mc], func=AF.Square
                        )
                    for mc in range(4):
                        nc.tensor.matmul(
                            gp, r(sw_sb[:, j, mc * 128:(mc + 1) * 128]),
                            r(sq[:, mc, :]), start=(mc == 0), stop=False,
                        )
                    nc.tensor.matmul(gp, r(eps_sb), r(gi2_cur), start=False, stop=True)
                    gi2_new = scr.tile([128, BLK], FP32R, tag="gi2", bufs=2)
                    nc.vector.tensor_copy(gi2_new, gp)
                    for mc in range(4):
                        if mc % 2 == 0:
                            nc.scalar.activation(
                                out=x_next[:, mc, :], in_=psums[mc], func=AF.Relu
                            )
                        else:
                            nc.vector.tensor_scalar_max(
                                out=x_next[:, mc, :], in0=psums[mc], scalar1=0.0
                            )
                    x_cur = x_next
                    gi2_cur = gi2_new

                # ---- output layer ----
                yp = ps_misc.tile([4, BLK], FP32, tag="mt")
                for kc in range(4):
                    nc.tensor.matmul(
                        yp, r(wout_sb[:, kc, :]), r(x_cur[:, kc, :]),
                        start=(kc == 0), stop=(kc == 3),
                    )
                ysb = scr.tile([4, BLK], FP32, tag="ysb")
                nc.vector.tensor_copy(ysb, yp)
                nc.sync.dma_start(out=yhat_dr[:, b * BLK:(b + 1) * BLK], in_=ysb)
                g4 = scr.tile([4, BLK], FP32, tag="g4")
                nc.vector.tensor_copy(g4, gi2_cur[0:4, :])
                nc.sync.dma_start(out=gi2_dr[:, b * BLK:(b + 1) * BLK], in_=g4)

            # ---- final: y = yhat / sqrt(gi2); out = [tanh(y0), 255*y1..3] ----
            # Two passes bounced through DRAM so the ACT table set only
            # switches twice (Sqrt set in pass A, back to the sin/tanh set).
            FB = min(512, s_core)
            for p in range(s_core // FB):
                sl = slice(p * FB, (p + 1) * FB)
                yh = finp.tile([4, FB], FP32, tag="yh", name="yh")
                gi = finp.tile([4, FB], FP32, tag="gi", name="gi")
                nc.sync.dma_start(out=yh, in_=yhat_dr[:, sl])
                nc.sync.dma_start(out=gi, in_=gi2_dr[:, sl])
                nc.scalar.activation(out=gi, in_=gi, func=AF.Sqrt)
                nc.vector.reciprocal(out=gi, in_=gi)
                nc.vector.tensor_mul(yh, yh, gi)
                nc.sync.dma_start(out=yhat_dr[:, sl], in_=yh)
            for p in range(s_core // FB):
                sl = slice(p * FB, (p + 1) * FB)
                yh = finp.tile([4, FB], FP32, tag="yh", name="yh")
                nc.sync.dma_start(out=yh, in_=yhat_dr[:, sl])
                nc.scalar.mul(out=yh, in_=yh, mul=255.0)
                nc.scalar.activation(
                    out=yh[0:1, :], in_=yh[0:1, :], func=AF.Tanh, scale=1.0 / 255.0
                )
                nc.sync.dma_start(out=outT_d[:, sl], in_=yh)

    nc.compile()
    return nc


def kernel(**inputs):
    if _general_case_needed(inputs):
        return _numpy_fallback(inputs)

    from concourse.bass_utils import run_bass_kernel_spmd

    pre = _precompute(inputs)
    inp = np.ascontiguousarray(np.asarray(inputs["input"], np.float32))

    if "nc" not in _NC_CACHE:
        _NC_CACHE["nc"] = _build_bass()
    nc = _NC_CACHE["nc"]

    in_maps = [
        {
            "inp": np.ascontiguousarray(inp[c * S_CORE:(c + 1) * S_CORE]),
            "w0p": pre["w0p"], "whp": pre["whp"], "swp": pre["swp"],
            "epst": pre["epst"], "ones_row": pre["ones_row"],
            "ident": pre["ident"], "gaussT": pre["gaussT"],
            "sel8": pre["sel8"], "woutp": pre["woutp"],
            "onesgi": pre["onesgi"],
        }
        for c in range(N_CORES)
    ]

    res = run_bass_kernel_spmd(
        nc, in_maps, core_ids=list(range(N_CORES)),
        trace=bool(int(os.environ.get("KERNEL_TRACE", "0"))),
    )
    kernel.last_results = res
    outs = [res.results[c]["outT"] for c in range(N_CORES)]
    return np.ascontiguousarray(
        np.concatenate([o.T for o in outs], axis=0).astype(np.float32)
    )


# revision 23
# speedup vs baseline: 77.5732x; 1.1310x over previous
"""Trainium2 Bass kernel for nn_Decoder (latent-grid decoder MLP).

Contract: kernel(**inputs) takes the FULL unsharded inputs (as produced by
setup_inputs()) and returns the FULL [65536, 4] float32 output. Internally the
65536 points are sharded across 8 NeuronCores (pure data parallel); the small
weights are replicated.

Algorithm (mathematically equivalent to the reference):
  - G=2 trilinear interp of a per-sample 2x2x2 grid always lands in cell
    (0,0,0) (indices clip to [0, G-2] = [0,0]), so
    lat_i = sum_m w_m(xyz) * (lat @ A_m), A_m = convT_w[:, :, di, dj, dk].
  - The interp + Fourier features + first MLP layer fold into one matmul:
    u = [w_0*lat, ..., w_7*lat, sin(2 pi ang), cos(2 pi ang)]  (2304 dims),
    h0 = u @ M0 with M0 = [A_stack @ W0_top; W0_sin; W0_cos] (host-folded).
  - LayerNorm mean-subtraction folds into the weights (column centering);
    ln gamma folds in too. The per-sample rstd is deferred via LN's positive
    scale invariance: activations stay unnormalized, and gi2 (squared inverse
    scale) follows gi2' = ssq_w/512 + eps*gi2, applied once at the end.
    Requires all biases and ln_b == 0 (true for this model; a numpy fallback
    covers the general case).
  - ssq_w and the eps*gi2 term are accumulated by the TensorEngine itself
    (weighted-ones stationary operands producing a broadcast [128, S] PSUM
    tile), so no partition reductions or per-sample row ops are needed.
Activations live in [feature, sample] layout; matmuls run as fp32r (full PE
rate at N=512).
"""

import os
import numpy as np

N_CORES = 8
N_TOTAL = 65536
S_CORE = N_TOTAL // N_CORES          # 8192 samples per core
BLK = 512                            # samples per block
N_BLOCKS = S_CORE // BLK             # 16
EPS = 1e-5
N_LAYERS = 8                         # LN+relu layers (layer0 + 7 hidden)


def _precompute(inputs):
    """Host-side weight folding. Returns dict of constant arrays (fp32)."""
    convT_w = np.asarray(inputs["convT_w"], np.float32)
    W0 = np.asarray(inputs["W0"], np.float32)
    Wh = np.asarray(inputs["Wh"], np.float32)
    ln_g = np.asarray(inputs["ln_g"], np.float32)
    gauss = np.asarray(inputs["gauss"], np.float32)
    W_out = np.asarray(inputs["W_out"], np.float32)

    # A_stack[m*256+i, c] = convT_w[i, c, di, dj, dk], m = 4*di + 2*dj + dk
    A_stack = convT_w.transpose(2, 3, 4, 0, 1).reshape(8 * 256, 512)
    M0 = np.concatenate([A_stack @ W0[:512], W0[512:640], W0[640:768]], axis=0)

    def center_scale(W, g):
        Wc = W - W.mean(axis=1, keepdims=True)
        return np.ascontiguousarray(Wc * g[None, :], np.float32)

    W_eff = [center_scale(M0, ln_g[0])] + [
        center_scale(Wh[l], ln_g[l + 1]) for l in range(7)
    ]
    # pack each layer's weights as [128, n_kchunks, 512]
    def pack(W):
        K = W.shape[0]
        kc = K // 128
        return W.reshape(kc, 128, 512).transpose(1, 0, 2).reshape(128, kc * 512)

    w0p = np.ascontiguousarray(pack(W_eff[0]))                       # [128, 18*512]
    whp = np.ascontiguousarray(
        np.concatenate([pack(W) for W in W_eff[1:]], axis=1))        # [128, 28*512]
    # stats lhsT, per layer j and feature chunk mc:
    # tile[k, mc*128 + m] = 1/(512 * g_j[mc*128+k]^2)  (replicated along m)
    sw_cols = []
    for j in (6, 7):
        swv = (1.0 / (512.0 * ln_g[j] ** 2)).astype(np.float32)
        t = np.empty((128, 512), np.float32)
        for mc in range(4):
            t[:, mc * 128:(mc + 1) * 128] = swv[mc * 128:(mc + 1) * 128, None]
        sw_cols.append(t)
    swp = np.ascontiguousarray(np.concatenate(sw_cols, axis=1))      # [128, 2*512]

    return {
        "w0p": w0p,
        "whp": whp,
        "swp": swp,
        "ident": np.eye(128, dtype=np.float32),
        "gaussT": np.ascontiguousarray(gauss.T.astype(np.float32)),  # [3, 128]
        "sel8": np.ascontiguousarray(
            np.kron(np.eye(8, dtype=np.float32), np.ones((1, 128), np.float32))),
        "woutp": np.ascontiguousarray(
            W_out.reshape(4, 128, 4).transpose(1, 0, 2).reshape(128, 16)),
    }


def _general_case_needed(inputs):
    z = lambda a: bool(np.all(np.asarray(a) == 0))
    return not (
        z(inputs["convT_b"]) and z(inputs["b0"]) and z(inputs["bh"])
        and z(inputs["ln_b"]) and z(inputs["b_out"])
        and bool(np.all(np.abs(np.asarray(inputs["ln_g"])) > 1e-3))
    )


def _numpy_fallback(inputs):
    """Reference in numpy (slow; only for inputs outside the fast path)."""
    inp = np.asarray(inputs["input"], np.float32)
    convT_w = np.asarray(inputs["convT_w"], np.float32)
    convT_b = np.asarray(inputs["convT_b"], np.float32)
    gauss = np.asarray(inputs["gauss"], np.float32)
    W0 = np.asarray(inputs["W0"], np.float32)
    b0 = np.asarray(inputs["b0"], np.float32)
    Wh = np.asarray(inputs["Wh"], np.float32)
    bh = np.asarray(inputs["bh"], np.float32)
    ln_g = np.asarray(inputs["ln_g"], np.float32)
    ln_b = np.asarray(inputs["ln_b"], np.float32)
    W_out = np.asarray(inputs["W_out"], np.float32)
    b_out = np.asarray(inputs["b_out"], np.float32)
    xyz = inp[:, -3:]
    lat = inp[:, :-3]
    f = (xyz + 1.0) * 0.5
    frac = f - np.clip(f.astype(np.int32), 0, 0)
    A = convT_w.transpose(2, 3, 4, 0, 1)
    lat_i = np.zeros((inp.shape[0], 512), np.float32)
    wx = [1 - frac[:, 0], frac[:, 0]]
    wy = [1 - frac[:, 1], frac[:, 1]]
    wz = [1 - frac[:, 2], frac[:, 2]]
    for di in (0, 1):
        for dj in (0, 1):
            for dk in (0, 1):
                w = (wx[di] * wy[dj] * wz[dk]).astype(np.float32)
                lat_i += (lat @ A[di, dj, dk]) * w[:, None]
    lat_i += convT_b[None, :]
    ang = 2 * np.pi * (xyz @ gauss.T)
    x = np.concatenate([lat_i, np.sin(ang), np.cos(ang)], axis=1)

    def ln(t, g, b):
        mu = t.mean(-1, keepdims=True)
        var = ((t - mu) ** 2).mean(-1, keepdims=True)
        return (t - mu) / np.sqrt(var + EPS) * g + b

    x = np.maximum(ln(x @ W0 + b0, ln_g[0], ln_b[0]), 0)
    for l in range(7):
        x = np.maximum(ln(x @ Wh[l] + bh[l], ln_g[l + 1], ln_b[l + 1]), 0)
    y = x @ W_out + b_out
    return np.concatenate([np.tanh(y[:, :1]), y[:, 1:] * 255.0], axis=1).astype(np.float32)


_NC_CACHE = {}


def _build_bass(s_core=S_CORE, repeat=1):
    """Build the per-core Bass module (SPMD; same program on all 8 cores)."""
    import concourse.bass as bass
    import concourse.bacc as bacc
    import concourse.tile as tile
    from concourse import mybir

    FP32 = mybir.dt.float32
    FP32R = mybir.dt.float32r
    AF = mybir.ActivationFunctionType
    ALU = mybir.AluOpType
    TWO_PI = float(2.0 * np.pi)
    MAGIC = 12582912.0            # 1.5 * 2^23: fp32 add/sub rounds to integer
    n_blocks = s_core // BLK

    nc = bacc.Bacc("TRN2", target_bir_lowering=False, debug=False)

    inp_d = nc.dram_tensor("inp", [s_core, 259], FP32, kind="ExternalInput").ap()
    w0p_d = nc.dram_tensor("w0p", [128, 18 * 512], FP32R, kind="ExternalInput").ap()
    whp_d = nc.dram_tensor("whp", [128, 28 * 512], FP32R, kind="ExternalInput").ap()
    swp_d = nc.dram_tensor("swp", [128, 2 * 512], FP32R, kind="ExternalInput").ap()
    ident_d = nc.dram_tensor("ident", [128, 128], FP32, kind="ExternalInput").ap()
    gaussT_d = nc.dram_tensor("gaussT", [3, 128], FP32R, kind="ExternalInput").ap()
    sel8_d = nc.dram_tensor("sel8", [8, 8 * 128], FP32R, kind="ExternalInput").ap()
    woutp_d = nc.dram_tensor("woutp", [128, 16], FP32R, kind="ExternalInput").ap()
    outT_d = nc.dram_tensor("outT", [4, s_core], FP32, kind="ExternalOutput").ap()

    def r(ap):
        return ap

    with tile.TileContext(nc) as tc:
        with (
            tc.tile_pool(name="const", bufs=1) as constp,
            tc.tile_pool(name="weights", bufs=1) as weightp,
            tc.tile_pool(name="inblk", bufs=2) as inp_pool,
            tc.tile_pool(name="acts", bufs=2) as actp,
            tc.tile_pool(name="scratch", bufs=2) as scr,
            tc.tile_pool(name="ps_t", bufs=1, space="PSUM") as ps_t,
            tc.tile_pool(name="ps_misc", bufs=2, space="PSUM") as ps_misc,
            tc.tile_pool(name="ps_gi", bufs=2, space="PSUM") as ps_gi,
        ):
            # ---- constants / weights (loaded once, resident; split into
            # chunked DMAs so they spread across queues and overlap) ----
            w0_sb = weightp.tile([128, 18, 512], FP32R)
            w0r = w0p_d.rearrange("p (c f) -> p c f", c=18)
            for ch in range(3):
                nc.sync.dma_start(
                    out=w0_sb[:, ch * 6:(ch + 1) * 6, :], in_=w0r[:, ch * 6:(ch + 1) * 6, :])
            wh_sb = weightp.tile([128, 28, 512], FP32R)
            whr = whp_d.rearrange("p (c f) -> p c f", c=28)
            for ch in range(4):
                nc.sync.dma_start(
                    out=wh_sb[:, ch * 7:(ch + 1) * 7, :], in_=whr[:, ch * 7:(ch + 1) * 7, :])
            sw_sb = weightp.tile([128, 2, 512], FP32R)
            nc.sync.dma_start(out=sw_sb, in_=swp_d.rearrange("p (c f) -> p c f", c=2))
            ident_dma = constp.tile([128, 128], FP32, name="ident_dma")
            nc.sync.dma_start(out=ident_dma, in_=ident_d)
            # DVE-gate the identity so PE transposes only ever wait on DVE
            ident_sb = constp.tile([128, 128], FP32, name="ident_sb")
            nc.vector.tensor_copy(ident_sb, ident_dma)
            gaussT_sb = constp.tile([3, 128], FP32R)
            nc.sync.dma_start(out=gaussT_sb, in_=gaussT_d)
            sel8_sb = constp.tile([8, 8, 128], FP32R)
            nc.sync.dma_start(out=sel8_sb, in_=sel8_d.rearrange("p (m f) -> p m f", m=8))
            wout_sb = weightp.tile([128, 4, 4], FP32R)
            nc.sync.dma_start(out=wout_sb, in_=woutp_d.rearrange("p (c f) -> p c f", c=4))

            inp_r = inp_d.rearrange("(b sc p) f -> b p sc f", sc=4, p=128)

            def load_block(b):
                """DMA a block in and DVE-gate it: downstream consumers then
                depend only on the DVE semaphore (PE LDW takes 1 wait max)."""
                inb0 = inp_pool.tile([128, 4, 259], FP32, tag="inb0", name="inb0")
                nc.sync.dma_start(out=inb0, in_=inp_r[b])
                inb = inp_pool.tile([128, 4, 259], FP32, tag="inb", name="inb")
                nc.vector.tensor_copy(inb, inb0)
                return inb

            for _rep in range(repeat):
              inb_next = load_block(0)
              for b in range(n_blocks):
                inb = inb_next

                # ---- transpose lat and xyz to [feature, sample] ----
                latT = scr.tile([128, 2, BLK], FP32, tag="latT", bufs=1)
                xyzT = scr.tile([3, BLK], FP32R, tag="xyzT")
                for sc in range(4):
                    for fc in range(2):
                        tp = ps_misc.tile([128, 128], FP32, tag="mt")
                        nc.tensor.transpose(
                            tp, inb[:, sc, fc * 128:(fc + 1) * 128], ident_sb
                        )
                        nc.vector.tensor_copy(latT[:, fc, sc * 128:(sc + 1) * 128], tp)
                    tp3 = ps_misc.tile([3, 128], FP32, tag="mt")
                    nc.tensor.transpose(tp3, inb[:, sc, 256:259], ident_sb)
                    nc.vector.tensor_copy(xyzT[:, sc * 128:(sc + 1) * 128], tp3)

                # ---- corner weights in sample layout (whole block), then transpose ----
                w8T = scr.tile([8, BLK], FP32R, tag="w8T", bufs=1)
                f3 = scr.tile([128, 4, 3], FP32, tag="f3")
                nc.vector.tensor_scalar(
                    out=f3, in0=inb[:, :, 256:259],
                    scalar1=0.5, scalar2=0.5, op0=ALU.mult, op1=ALU.add,
                )
                om3 = scr.tile([128, 4, 3], FP32, tag="om3")
                nc.vector.tensor_scalar(
                    out=om3, in0=f3, scalar1=1.0, scalar2=-1.0,
                    op0=ALU.subtract, op1=ALU.mult,
                )
                wxy = scr.tile([128, 4, 4], FP32, tag="wxy")
                w8s = scr.tile([128, 4, 8], FP32, tag="w8s")
                for di in (0, 1):
                    xs = (f3 if di else om3)[:, :, 0:1]
                    for dj in (0, 1):
                        ys = (f3 if dj else om3)[:, :, 1:2]
                        nc.vector.tensor_tensor(
                            out=wxy[:, :, di * 2 + dj:di * 2 + dj + 1],
                            in0=xs, in1=ys, op=ALU.mult,
                        )
                for m in range(8):
                    di, dj, dk = (m >> 2) & 1, (m >> 1) & 1, m & 1
                    zsl = (f3 if dk else om3)[:, :, 2:3]
                    nc.vector.tensor_tensor(
                        out=w8s[:, :, m:m + 1],
                        in0=wxy[:, :, di * 2 + dj:di * 2 + dj + 1],
                        in1=zsl, op=ALU.mult,
                    )
                for sc in range(4):
                    tp8 = ps_misc.tile([8, 128], FP32, tag="mt")
                    nc.tensor.transpose(tp8, w8s[:, sc, :], ident_sb)
                    nc.vector.tensor_copy(w8T[:, sc * 128:(sc + 1) * 128], tp8)

                if b + 1 < n_blocks:
                    inb_next = load_block(b + 1)

                # ---- fourier angle, range-reduced to [-0.5, 0.5] ----
                angp = ps_misc.tile([128, BLK], FP32, tag="mt")
                nc.tensor.matmul(angp, r(gaussT_sb), r(xyzT), start=True, stop=True)
                ang_sb = scr.tile([128, BLK], FP32, tag="rr", bufs=3, name="ang_sb")
                nc.vector.tensor_copy(ang_sb, angp)
                # zs = ang - round(ang); zc = a25 - round(a25), a25 = ang + 0.25
                zs_r = scr.tile([128, BLK], FP32, tag="rr", bufs=3, name="zs_r")
                nc.vector.tensor_scalar(
                    out=zs_r, in0=ang_sb, scalar1=MAGIC, scalar2=MAGIC,
                    op0=ALU.add, op1=ALU.subtract,
                )
                zs = scr.tile([128, BLK], FP32, tag="zs", bufs=1)
                nc.vector.tensor_sub(zs, ang_sb, zs_r)
                a25 = scr.tile([128, BLK], FP32, tag="a25", bufs=1)
                nc.vector.tensor_scalar_add(out=a25, in0=ang_sb, scalar1=0.25)
                zc_r = scr.tile([128, BLK], FP32, tag="rr", bufs=3, name="zc_r")
                nc.vector.tensor_scalar(
                    out=zc_r, in0=a25, scalar1=MAGIC, scalar2=MAGIC,
                    op0=ALU.add, op1=ALU.subtract,
                )
                zc = scr.tile([128, BLK], FP32, tag="zc", bufs=1)
                nc.vector.tensor_sub(zc, a25, zc_r)

                # ---- layer 0: build u chunks incrementally + matmul ----
                psums = [ps_t.tile([128, BLK], FP32, tag=f"pt{mc}", name=f"pt{mc}") for mc in range(4)]
                uch_i = 0

                def l0_accum(u_ap, last=False):
                    nonlocal uch_i
                    for mc in range(4):
                        nc.tensor.matmul(
                            psums[mc],
                            r(w0_sb[:, uch_i, mc * 128:(mc + 1) * 128]),
                            r(u_ap),
                            start=(uch_i == 0), stop=last,
                        )
                    uch_i += 1

                for m in range(8):
                    bc = ps_misc.tile([128, BLK], FP32, tag="mt")
                    nc.tensor.matmul(
                        bc, r(sel8_sb[:, m, :]), r(w8T), start=True, stop=True
                    )
                    for kc in range(2):
                        uch = scr.tile([128, BLK], FP32R, tag="uch")
                        nc.vector.tensor_tensor(
                            out=uch, in0=latT[:, kc, :], in1=bc, op=ALU.mult
                        )
                        l0_accum(uch)
                ffs = scr.tile([128, BLK], FP32R, tag="uch")
                nc.scalar.activation(out=ffs, in_=zs, func=AF.Sin, scale=TWO_PI)
                l0_accum(ffs)
                ffc = scr.tile([128, BLK], FP32R, tag="uch")
                nc.scalar.activation(out=ffc, in_=zc, func=AF.Sin, scale=TWO_PI)
                l0_accum(ffc, last=True)

                # ---- LN layers: relu; stats only for the last two.
                # Stats matmuls are emitted AFTER the next layer's main
                # matmuls so the PE never waits on the ACT squares.
                x_cur = None
                pending_stats = None
                gp6 = gp7 = None

                def emit_stats():
                    nonlocal gp6, gp7, pending_stats
                    if pending_stats is None:
                        return
                    jj, sq_t = pending_stats
                    gp = ps_gi.tile([128, BLK], FP32, tag="gp", name=f"gp{jj}")
                    for mc in range(4):
                        nc.tensor.matmul(
                            gp, r(sw_sb[:, jj - 6, mc * 128:(mc + 1) * 128]),
                            r(sq_t[:, mc, :]), start=(mc == 0), stop=(mc == 3),
                        )
                    if jj == 6:
                        gp6 = gp
                    else:
                        gp7 = gp
                    pending_stats = None

                for j in range(N_LAYERS):
                    if j > 0:
                        psums = [
                            ps_t.tile([128, BLK], FP32, tag=f"pt{mc}", name=f"pt{mc}")
                            for mc in range(4)
                        ]
                        for mc in range(4):
                            for kc in range(4):
                                nc.tensor.matmul(
                                    psums[mc],
                                    r(wh_sb[:, (j - 1) * 4 + kc,
                                            mc * 128:(mc + 1) * 128]),
                                    r(x_cur[:, kc, :]),
                                    start=(kc == 0), stop=(kc == 3),
                                )
                    emit_stats()
                    x_next = actp.tile([128, 4, BLK], FP32R, tag="xn")
                    for mc in range(4):
                        nc.scalar.activation(
                            out=x_next[:, mc, :], in_=psums[mc], func=AF.Relu
                        )
                    if j >= 6:
                        sq = scr.tile([128, 4, BLK], FP32R, tag="sq", bufs=2)
                        for mc in range(4):
                            nc.scalar.activation(
                                out=sq[:, mc, :], in_=psums[mc], func=AF.Square
                            )
                        pending_stats = (j, sq)
                    x_cur = x_next

                # ---- output layer ----
                yp = ps_gi.tile([4, BLK], FP32, tag="gp", name="yp")
                for kc in range(4):
                    nc.tensor.matmul(
                        yp, r(wout_sb[:, kc, :]), r(x_cur[:, kc, :]),
                        start=(kc == 0), stop=(kc == 3),
                    )
                emit_stats()

                # gi2 = gp7 + eps*gp6, then finalize this block in place:
                # out = [tanh(yhat/gi), 255*yhat/gi] with gi = sqrt(gi2)
                g6 = scr.tile([4, BLK], FP32, tag="g6")
                nc.vector.tensor_copy(g6, gp6[0:4, :])
                g4 = scr.tile([4, BLK], FP32, tag="g4")
                nc.vector.scalar_tensor_tensor(
                    out=g4, in0=g6, scalar=EPS, in1=gp7[0:4, :],
                    op0=ALU.mult, op1=ALU.add,
                )
                sg = scr.tile([4, BLK], FP32, tag="sg")
                nc.scalar.activation(out=sg, in_=g4, func=AF.Sqrt)
                rg = scr.tile([4, BLK], FP32, tag="rg")
                nc.vector.reciprocal(out=rg, in_=sg)
                yv = scr.tile([4, BLK], FP32, tag="yv")
                nc.vector.tensor_tensor(out=yv, in0=yp, in1=rg, op=ALU.mult)
                nc.scalar.mul(out=yv, in_=yv, mul=255.0)
                nc.scalar.activation(
                    out=yv[0:1, :], in_=yv[0:1, :], func=AF.Tanh, scale=1.0 / 255.0
                )
                nc.sync.dma_start(out=outT_d[:, b * BLK:(b + 1) * BLK], in_=yv)

    nc.compile()
    return nc


def kernel(**inputs):
    if _general_case_needed(inputs):
        return _numpy_fallback(inputs)

    from concourse.bass_utils import run_bass_kernel_spmd

    pre = _precompute(inputs)
    inp = np.ascontiguousarray(np.asarray(inputs["input"], np.float32))

    if "nc" not in _NC_CACHE:
        _NC_CACHE["nc"] = _build_bass()
    nc = _NC_CACHE["nc"]

    in_maps = [
        {
            "inp": np.ascontiguousarray(inp[c * S_CORE:(c + 1) * S_CORE]),
            "w0p": pre["w0p"], "whp": pre["whp"], "swp": pre["swp"],
            "ident": pre["ident"], "gaussT": pre["gaussT"],
            "sel8": pre["sel8"], "woutp": pre["woutp"],
        }
        for c in range(N_CORES)
    ]

    res = run_bass_kernel_spmd(
        nc, in_maps, core_ids=list(range(N_CORES)),
        trace=bool(int(os.environ.get("KERNEL_TRACE", "0"))),
    )
    kernel.last_results = res
    outs = [res.results[c]["outT"] for c in range(N_CORES)]
    return np.ascontiguousarray(
        np.concatenate([o.T for o in outs], axis=0).astype(np.float32)
    )
